# revision 43
# baseline (speedup 1.0000x reference)
# GCN layer kernel for Trainium2: out[b] = relu((a[b] @ x[b]) @ W) * mask[b]
#
# Sharding: data-parallel over the batch (graph) dim. B=8 graphs, 8 cores,
# one graph per core; W replicated. Inputs are the FULL tensors; shards are
# sliced host-side and the per-core outputs stacked back together.
#
# Per-core dataflow (a: [2048,2048], x: [2048,512], W: [512,512]):
#   - All matmul operands are bf16 (same PE rate as f32r; rel-err ~3e-3 vs
#     the 2e-2 gate). a and W are cast fp32->bf16 during the DMA load
#     itself (SWDGE cast-on-DMA), x is cast on the mostly-idle DVE.
#   - a must be contracted over its column index m, which requires aT with
#     m on the partition axis. aT chunks live in tiles
#       at[p, k, mi, j] = a[512*nj + 128k + j, 128mi + p]
#     and mm1's rhs for (nj, mi) is at[nj][:, :, mi, :] ([128, 4, 128] =
#     the chunk's 512 n-columns in order).
#     They are produced two ways (hybrid, balancing PE vs DMA):
#       * chunks 0-1: PE transpose-mode in bf16 (FWL halves the weight
#         load vs fp32) while the DMA engines are busy streaming a in;
#         PSUM->SBUF copybacks alternate DVE/ACT.
#       * chunks 2-3: DMA xbar transpose (SBUF->SBUF, one instruction per
#         chunk) which runs as the a-load stream drains, off the PE.
#   - t^T[f,n] = sum_m x[m,f] * aT[m,n]:  lhsT = x tile, rhs = aT slice.
#   - out[n,d] = sum_f t^T[f,n] * W[f,d]: lhsT = t^T tile, rhs = W, which
#     lands out in [n,d] layout; 4 row-tiles batch into one 1MB store.
#   - mask[n] = any(x[n,:] != 0) via sum(|x|) > 0 on ACT, applied fused
#     into the ReLU (relu(mask * t) == mask * relu(t)).
#
# DMA schedule (learned from traces):
#   - Few, BIG DMAs: the tile scheduler chains DMAs through a small
#     completion-semaphore pool and each link costs ~2us receipt latency.
#   - SWDGE (gpsimd) emission is non-blocking and its descriptors are
#     consumed in order, so all a-group loads queue up front and stream
#     continuously at HBM rate; group 0 is split in half so chunk 0's
#     compute starts earlier (mm1 chunk 0 runs as two 256-wide halves).
#   - HWDGE DMA instructions block their engine while the ring is full:
#     sync carries the x loads (early) and the 2 xbar transposes (late);
#     scalar/ACT stays DMA-free so copybacks and activations flow; stores
#     ride gpsimd (emission waits on the ReLU data; engine idle by then).

import numpy as np

B, N, F, D = 8, 2048, 512, 512
P = 128
NT = N // P        # 16 row-tiles of n (and of m, since a is square)
FT = F // P        # 4 tiles of f
NCHUNK = 512       # n is processed in chunks of 512 columns
NJ = N // NCHUNK   # 4
NSUB = NCHUNK // P # 4
PE_CHUNKS = 2      # chunks transposed on the PE; the rest use the DMA xbar

_CACHE = {}


def _build_nc():
    from contextlib import ExitStack

    from concourse import bacc, mybir, tile
    from concourse.masks import make_identity

    f32 = mybir.dt.float32
    bf16 = mybir.dt.bfloat16
    AF = mybir.ActivationFunctionType

    nc = bacc.Bacc(None)
    # deeper SWDGE descriptor ring: fewer emission-pacing DRAIN stalls in
    # the load stream (default 16KB ring wraps ~10x over our ~4600
    # descriptors)
    nc.dynamic_dma_scratch_size = 24576
    a_d = nc.dram_tensor("a", [N, N], f32, kind="ExternalInput")
    x_d = nc.dram_tensor("x", [N, F], f32, kind="ExternalInput")
    w_d = nc.dram_tensor("kernel", [F, D], f32, kind="ExternalInput")
    o_d = nc.dram_tensor("out", [N, D], f32, kind="ExternalOutput")

    with tile.TileContext(nc) as tc, ExitStack() as ctx:
        const = ctx.enter_context(tc.tile_pool(name="const", bufs=1))
        xp = ctx.enter_context(tc.tile_pool(name="xp", bufs=1))
        wp = ctx.enter_context(tc.tile_pool(name="wp", bufs=1))
        abh = ctx.enter_context(tc.tile_pool(name="abh", bufs=2))
        abp = ctx.enter_context(tc.tile_pool(name="abp", bufs=3))
        atp = ctx.enter_context(tc.tile_pool(name="atp", bufs=NJ))
        ttp = ctx.enter_context(tc.tile_pool(name="ttp", bufs=2))
        outp = ctx.enter_context(tc.tile_pool(name="outp", bufs=2))
        scr = ctx.enter_context(tc.tile_pool(name="scr", bufs=2))
        ps_mm = ctx.enter_context(tc.tile_pool(name="ps_mm", bufs=4, space="PSUM"))
        ps_o = ctx.enter_context(tc.tile_pool(name="ps_o", bufs=2, space="PSUM"))
        ps_tp = ctx.enter_context(tc.tile_pool(name="ps_tp", bufs=2, space="PSUM"))

        ident = const.tile([P, P], bf16)
        make_identity(nc, ident[:])
        junk = const.tile([P, NCHUNK], bf16)
        nc.gpsimd.memset(junk[:], 0.0)

        def warm(n, rhs=None):
            # bf16 identity matmuls: register as HAM activity, output unused.
            # Borrow the mm2 PSUM slots (idle during the start-up window).
            for _ in range(n):
                r = ident[:] if rhs is None else rhs
                nfree = 1
                for s in r.shape[1:]:
                    nfree *= s
                pw = ps_o.tile([P, D], f32, tag="pso", name="pw")
                nc.tensor.matmul(
                    pw[:, :nfree], lhsT=ident[:], rhs=r, start=True, stop=True
                )

        warm(28, rhs=junk[:])

        # a loads on gpsimd (SWDGE cast fp32->bf16), consumed in order:
        # group 0 as two 2-strip halves, then three 4-strip groups.
        # ab[p, k, m] = a[nbase + 128k + p, m]
        def load_a(nbase_rows, nrows, tag, pool, name):
            t = pool.tile([P, nrows // P, N], bf16, tag=tag, name=name)
            nc.gpsimd.dma_start(
                t[:],
                a_d[nbase_rows : nbase_rows + nrows, :].rearrange(
                    "(k p) m -> p k m", p=P
                ),
            )
            return t

        # x rides Q0 (SWDGE cast-load): FIFO position on the one streaming
        # queue is the only reliable way to get its data on time -- any
        # side-queue load is starved to a few percent of bandwidth while
        # the a-stream runs. ONE full 4MB load (2048 full-row 2KB
        # descriptors) instead of column chunks: half the descriptor count
        # means less of the DMA-engine-15 descriptor-ring lag that delays
        # every later load's completion semaphore.
        x_sb = xp.tile([P, NT, F], bf16)

        ab0a = load_a(0, 2 * P, "abh", abh, "ab0a")
        warm(3, rhs=ab0a[:, 0, 0:NCHUNK])
        nc.gpsimd.dma_start(
            x_sb[:], x_d[:].rearrange("(o p) f -> p o f", p=P)
        )
        ab0b = load_a(2 * P, 2 * P, "abh", abh, "ab0b")
        warm(3, rhs=ab0a[:, 1, 0:NCHUNK])

        # w: SWDGE cast-load next; lands well before the first mm2 needs it.
        w_sb = wp.tile([P, FT, D], bf16)
        nc.gpsimd.dma_start(w_sb[:], w_d[:].rearrange("(o p) d -> p o d", p=P))

        ab = [None] * NJ
        for g in range(1, NJ):
            ab[g] = load_a(g * NCHUNK, NCHUNK, "ab", abp, f"ab{g}")

        at = [
            atp.tile([P, NJ, NT, P], bf16, tag="at", name=f"at{nj}")
            for nj in range(NJ)
        ]

        cb = 0

        def copyback(dst, src, eng=None):
            nonlocal cb
            if eng is None:
                eng = "v" if cb % 2 == 0 else "s"
                cb += 1
            if eng == "v":
                nc.vector.tensor_copy(dst, src)
            else:
                nc.scalar.copy(dst, src)

        def pe_transpose_strip(nj, k, src, eng=None):
            # 16 PE transposes of one 128-row strip, 4 PSUM quads:
            # at[nj][:, k, mi, j] = a_strip[j, 128mi + p]
            for q in range(NT // 4):
                ps = ps_tp.tile([P, NCHUNK], bf16, tag="pst", name="ps")
                for t in range(4):
                    mi = q * 4 + t
                    nc.tensor.transpose(
                        ps[:, t * P : (t + 1) * P],
                        src[:, mi * P : (mi + 1) * P],
                        ident[:],
                    )
                copyback(
                    at[nj][:, k, q * 4 : (q + 1) * 4, :],
                    ps[:].rearrange("p (q j) -> p q j", q=4),
                    eng=eng,
                )

        # chunk 0 (strips from the two half-group loads) and chunk 1 are
        # transposed on the PE while the DMA engines stream the loads.
        for k in range(2):
            pe_transpose_strip(0, k, ab0a[:, k], eng="v")

        # chunks 2..: one xbar transpose per chunk on the sync queue
        # (runs as the a-load stream drains): at[p,(k,mi),j] = ab[j,(k,..)]
        for nj in range(PE_CHUNKS, NJ):
            nc.sync.dma_start(at[nj][:], ab[nj][:], transpose=True)

        # mask accumulators; the per-row-tile |x| reductions ride along
        # inside chunk 0's mm1 phase.
        sumabs = const.tile([P, NT], f32)
        mask_sb = const.tile([P, NT], f32)

        for nj in range(NJ):
            tt_sb = ttp.tile([P, FT, NCHUNK], bf16, tag="tt", name=f"tt{nj}")
            if nj == 0:
                # two 256-wide halves, half-outer: each half's 4 fi passes
                # run as soon as that half-group's transposes land.
                pts = [
                    ps_mm.tile([P, NCHUNK], f32, tag="psm", name=f"pt_0_{fi}")
                    for fi in range(FT)
                ]
                for h in range(2):
                    if h == 1:
                        # second half-group's transposes ride between the
                        # two x-gated half passes
                        for k in range(2):
                            pe_transpose_strip(0, 2 + k, ab0b[:, k], eng="v")
                    else:
                        # x-wait filler: tied to the freshly transposed at0
                        # columns so the scheduler keeps it inside the gap,
                        # keeping the HAM clock-gate open (an idle gap
                        # >3.4us re-throttles the PE to 1.2GHz)
                        warm(48, rhs=at[0][:, 0, 0:2, :])
                    for fi in range(FT):
                        sl = pts[fi][:, h * 256 : (h + 1) * 256]
                        for mi in range(NT):
                            nc.tensor.matmul(
                                sl,
                                lhsT=x_sb[:, mi, fi * P : (fi + 1) * P],
                                rhs=at[0][:, 2 * h : 2 * h + 2, mi, :],
                                start=(mi == 0),
                                stop=(mi == NT - 1),
                            )
                        if h == 1:
                            for ni in range(fi * 4, fi * 4 + 4):
                                abs_scr = scr.tile([P, F], f32, tag="abs_scr")
                                nc.scalar.activation(
                                    abs_scr[:],
                                    x_sb[:, ni, :],
                                    AF.Abs,
                                    accum_out=sumabs[:, ni : ni + 1],
                                )
                            copyback(tt_sb[:, fi], pts[fi][:])
            elif nj == 1:
                # chunk 1's PE transposes weave into fi0's accumulation,
                # q-major with a 2-group pipeline offset: each mi-group's
                # rhs quads complete one group ahead, and the mm1 matmuls
                # between quad groups absorb the PSUM->SBUF copyback hops
                # that used to stall the 2-slot ps_tp pool.
                def q_quads(q):
                    for k in range(NJ):
                        ps = ps_tp.tile([P, NCHUNK], bf16, tag="pst", name="ps")
                        for t in range(4):
                            mi = q * 4 + t
                            nc.tensor.transpose(
                                ps[:, t * P : (t + 1) * P],
                                ab[1][:, k, mi * P : (mi + 1) * P],
                                ident[:],
                            )
                        copyback(
                            at[1][:, k, q * 4 : (q + 1) * 4, :],
                            ps[:].rearrange("p (q j) -> p q j", q=4),
                        )

                for fi in range(FT):
                    pt = ps_mm.tile([P, NCHUNK], f32, tag="psm", name=f"pt_{nj}_{fi}")
                    for grp in range(4):
                        if fi == 0:
                            if grp == 0:
                                q_quads(0)
                                q_quads(1)
                            elif grp < 3:
                                q_quads(grp + 1)
                        for mi in range(grp * 4, grp * 4 + 4):
                            nc.tensor.matmul(
                                pt[:],
                                lhsT=x_sb[:, mi, fi * P : (fi + 1) * P],
                                rhs=at[nj][:, :, mi, :],
                                start=(mi == 0),
                                stop=(mi == NT - 1),
                            )
                    copyback(tt_sb[:, fi], pt[:])
            else:
                for fi in range(FT):
                    pt = ps_mm.tile([P, NCHUNK], f32, tag="psm", name=f"pt_{nj}_{fi}")
                    for mi in range(NT):
                        nc.tensor.matmul(
                            pt[:],
                            lhsT=x_sb[:, mi, fi * P : (fi + 1) * P],
                            rhs=at[nj][:, :, mi, :],
                            start=(mi == 0),
                            stop=(mi == NT - 1),
                        )
                    copyback(tt_sb[:, fi], pt[:])
            if nj == 0:
                nc.vector.tensor_scalar(
                    mask_sb[:], sumabs[:], 0.0, None, mybir.AluOpType.is_gt
                )

            # out rows for this chunk: accumulate over the 4 f-tiles, then
            # fused relu+mask on ACT; 4 row-tiles batch into one 1MB store
            # emitted from the (by now idle) gpsimd SWDGE queue.
            last = nj == NJ - 1
            ob = outp.tile([P, NSUB, D], f32, tag="ob", name=f"ob{nj}")
            for ns in range(NSUB):
                # final chunk: borrow the (now idle) mm1 PSUM slots so the
                # four mm2 tiles don't chain on relu completions through
                # the 2-slot pso pool (each link costs ~1.5us of
                # cross-engine semaphore latency on the critical tail)
                if last:
                    po = ps_mm.tile([P, D], f32, tag="psm", name=f"po_{nj}_{ns}")
                else:
                    po = ps_o.tile([P, D], f32, tag="pso", name=f"po_{nj}_{ns}")
                for fi in range(FT):
                    nc.tensor.matmul(
                        po[:],
                        lhsT=tt_sb[:, fi, ns * P : (ns + 1) * P],
                        rhs=w_sb[:, fi],
                        start=(fi == 0),
                        stop=(fi == FT - 1),
                    )
                ni = nj * NSUB + ns
                nc.scalar.activation(
                    ob[:, ns], po[:], AF.Relu, scale=mask_sb[:, ni : ni + 1]
                )
                if last:
                    # store each row-tile as soon as its ReLU lands
                    nc.gpsimd.dma_start(o_d[ni * P : (ni + 1) * P, :], ob[:, ns])
            if not last:
                nc.gpsimd.dma_start(
                    o_d[nj * NCHUNK : (nj + 1) * NCHUNK, :].rearrange(
                        "(k p) d -> p k d", p=P
                    ),
                    ob[:],
                )

    nc.compile()
    return nc


def get_nc():
    if "nc" not in _CACHE:
        _CACHE["nc"] = _build_nc()
    return _CACHE["nc"]


def kernel(**inputs) -> np.ndarray:
    from concourse.bass_utils import run_bass_kernel_spmd

    x = np.ascontiguousarray(np.asarray(inputs["x"], dtype=np.float32))
    a = np.ascontiguousarray(np.asarray(inputs["a"], dtype=np.float32))
    w = np.ascontiguousarray(np.asarray(inputs["kernel"], dtype=np.float32))
    assert x.shape == (B, N, F) and a.shape == (B, N, N) and w.shape == (F, D)

    nc = get_nc()
    in_maps = [{"a": a[b], "x": x[b], "kernel": w} for b in range(B)]
    res = run_bass_kernel_spmd(nc, in_maps, core_ids=list(range(B)))
    return np.stack([res.results[b]["out"] for b in range(B)], axis=0)
